# revision 2
# baseline (speedup 1.0000x reference)
"""Trainium2 Bass kernel for nn_CrossNetwork (DCN-v1 cross network), v3.

Math: reference computes x_{i+1} = input * (x_i . w_i) + x_i + b_i, L=6 layers.
Writing x_i = input * c_i + B_i with B_i = sum_{j<i} b_j (a constant row
vector) and c_i a per-row scalar, the recursion collapses to
    u_i    = input . w_i                     (per row, one tall-skinny matmul)
    beta_i = B_i . w_i                       (host-computed constants)
    c_{i+1} = c_i * (1 + u_i) + beta_i ; c_0 = 1
    out    = input * c_L + B_L
For the b == 0 case this is out = input * prod_i(1 + u_i).

v3: software-pipelined double buffering across loop passes with monolithic
8MB load/store DMAs (ring order L_A S_B L_B S_A -> long same-direction runs,
minimal read/write turnaround, near-peak HBM BW); bf16 matmuls (1 cyc/row);
PSUM->SBUF copies on ACT, scales + scalar chain on DVE, no gpsimd.
"""

import numpy as np

import concourse.bass as bass
import concourse.mybir as mybir
import concourse.tile as tile
from concourse.bass_utils import run_bass_kernel_spmd
from concourse.masks import make_identity
from concourse.vector_clock import ScopedClock

F32 = mybir.dt.float32
BF16 = mybir.dt.bfloat16

B, D, L = 16384, 1024, 6
NCORES = 8
R = B // NCORES  # rows per core (2048)
P = 128
NCH = R // P  # chunks of 128 rows per core (16)
KB = D // P  # 128-wide k blocks (8)
GRP = 4  # chunks per psum accumulation group
NG = NCH // GRP  # 4 groups

MM_DT = BF16  # matmul operand dtype (bf16: 1 cyc/row vs fp32's 4)


def _patch_tile_drain():
    """This walrus build rejects >1 sem wait on a CTRL (Drain) instruction.

    Tile's kernel-tail drain waits on every sem domain at once; split it into
    chained single-wait drains.
    """
    if getattr(tile.TileContext, "_drain_patched", False):
        return

    def _drain_and_barrier(self, tick_clock, wait_clock):
        gc = tick_clock.global_clock
        entries = [(proc, t) for proc, t in enumerate(gc) if t > 0]
        if not entries:
            self.nc.sync.drain()
        for proc, t in entries:
            sub = ScopedClock()
            sub.require_at_least(None, proc, t)
            drain_inst = self.nc.sync.drain()
            wait_clock.add_sem_waits(drain_inst.ins, sub)

        self.nc.all_engine_barrier()
        assert self.sems is not None
        popped = self.nc._tile_sem_poison_stack.pop()
        assert popped is self._sem_poison
        self.nc.clear_and_free_semaphores(list(self.sems.allocated().values()))

    tile.TileContext._drain_and_barrier = _drain_and_barrier
    tile.TileContext._drain_patched = True


def _build(with_bias: bool, loop_n: int = 1, mode: str = "full"):
    nc = bass.Bass("TRN2")
    x_d = nc.dram_tensor("x", [R, D], F32, kind="ExternalInput")
    wt_d = nc.dram_tensor("wt", [D, L], F32, kind="ExternalInput")
    if with_bias:
        bl_d = nc.dram_tensor("bl", [1, D], F32, kind="ExternalInput")
        beta_d = nc.dram_tensor("beta", [1, L], F32, kind="ExternalInput")
    y_d = nc.dram_tensor("y", [R, D], F32, kind="ExternalOutput")

    # row = p*NCH + n ; whole-slab load: per-partition 64KB contiguous
    xv = x_d.rearrange("(p n) d -> p n d", p=P)  # [128, NCH, D]
    yv = y_d.rearrange("(p n) d -> p n d", p=P)
    wtv = wt_d.rearrange("(k p) s -> p k s", p=P)  # [128, KB, L]

    with tile.TileContext(nc) as tc:
        with (
            tc.tile_pool(name="consts", bufs=1) as consts,
            tc.tile_pool(name="xbuf", bufs=1) as xpool,
            tc.tile_pool(name="xt", bufs=6) as xtpool,
            tc.tile_pool(name="small", bufs=2 * GRP) as small,
            tc.tile_pool(name="pxt", bufs=4, space="PSUM") as pxt,
            tc.tile_pool(name="pu", bufs=2, space="PSUM") as pu,
        ):
            ident = consts.tile([P, P], F32)
            make_identity(nc, ident)
            ident6 = consts.tile([L, L], F32)
            make_identity(nc, ident6)
            wt_sb = consts.tile([P, KB, L], F32)
            nc.sync.dma_start(out=wt_sb, in_=wtv)
            wt_mm = wt_sb
            if MM_DT is not F32:
                wt_mm = consts.tile([P, KB, L], MM_DT, name="wt_mm")
                nc.vector.tensor_copy(wt_mm, wt_sb)
            if with_bias:
                bl_sb = consts.tile([P, D], F32)
                nc.sync.dma_start(
                    out=bl_sb,
                    in_=bass.AP(tensor=bl_d, offset=0, ap=[[0, P], [1, D]]),
                )
                beta_sb = consts.tile([P, L], F32)
                nc.sync.dma_start(
                    out=beta_sb,
                    in_=bass.AP(tensor=beta_d, offset=0, ap=[[0, P], [1, L]]),
                )
            bl = locals().get("bl_sb")
            beta = locals().get("beta_sb")

            args = (nc, tc, xtpool, small, pxt, pu, ident, ident6, wt_mm,
                    bl, beta, with_bias)

            if loop_n > 1:
                xa = xpool.tile([P, NCH, D], F32, tag="xa", name="xa")
                xb = xpool.tile([P, NCH, D], F32, tag="xb", name="xb")
                # pipeline prologue: pre-fill B so the first S_B is a
                # well-defined (if unscaled) store; overwritten in-iteration.
                nc.sync.dma_start(out=xb, in_=xv)
                with tc.For_i(0, loop_n, 1):
                    # ring: L_A  S_B  L_B  S_A  (two passes per trip)
                    nc.sync.dma_start(out=xa, in_=xv)
                    if mode != "dma":
                        _compute(xa, *args)
                    nc.sync.dma_start(out=yv, in_=xb)
                    nc.sync.dma_start(out=xb, in_=xv)
                    if mode != "dma":
                        _compute(xb, *args)
                    nc.sync.dma_start(out=yv, in_=xa)
            else:
                xa = xpool.tile([P, NCH, D], F32, tag="xa", name="xa")
                nc.sync.dma_start(out=xa, in_=xv)
                _compute(xa, *args)
                nc.sync.dma_start(out=yv, in_=xa)
    return nc


def _compute(x_sb, nc, tc, xtpool, small, pxt, pu, ident, ident6, wt_mm,
             bl_sb, beta_sb, with_bias):
    def chunk(n):  # chunk n -> AP [P, D]
        return x_sb[:, n, :]

    for g in range(NG):
        # U^T[6, 512] accumulated over k blocks; stationary weights
        # are only 6 columns so LDWEIGHTS is trivial.
        # Two phases of (16 transposes -> 4 matmuls) so the PE FIFO never
        # stalls waiting on a PSUM->SBUF copy: copy k completes while the
        # transposes for k+1..k+3 stream.
        ut_ps = pu.tile([L, GRP * P], F32, tag="u", name=f"ut{g}")
        for ph in range(2):
            xts = []
            for kk in range(KB // 2):
                k = ph * (KB // 2) + kk
                pxt_t = pxt.tile([P, GRP * P], F32, tag="pxt")
                for j in range(GRP):
                    nc.tensor.transpose(
                        pxt_t[:, j * P : (j + 1) * P],
                        chunk(g * GRP + j)[:, k * P : (k + 1) * P],
                        ident,
                    )
                xt_t = xtpool.tile([P, GRP * P], MM_DT, tag="xt")
                nc.scalar.copy(xt_t, pxt_t)
                xts.append((k, xt_t))
            for k, xt_t in xts:
                nc.tensor.matmul(
                    ut_ps[:],
                    wt_mm[:, k, :],
                    xt_t[:],
                    start=(k == 0),
                    stop=(k == KB - 1),
                )
        # 1 + U^T on DVE while copying PSUM->SBUF, then transpose
        # [6,128] blocks back to row-major [128,6] per chunk.
        u1t_t = xtpool.tile([L, GRP * P], F32, tag="u1t")
        nc.vector.tensor_scalar_add(u1t_t, ut_ps, 1.0)
        uj_ps = pu.tile([P, GRP, L], F32, tag="uj", name=f"uj{g}")
        for j in range(GRP):
            nc.tensor.transpose(
                uj_ps[:, j, :],
                u1t_t[:, j * P : (j + 1) * P],
                ident6,
            )
        if not with_bias:
            # group-batched: c = prod over the 6 (1+u_i) for all 4 chunks
            u1g_t = small.tile([P, GRP, L], F32, tag="u1g")
            nc.vector.tensor_copy(u1g_t, uj_ps)
            p3_t = small.tile([P, GRP, 3], F32, tag="p3")
            nc.vector.tensor_mul(p3_t, u1g_t[:, :, 0:3], u1g_t[:, :, 3:6])
            p1_t = small.tile([P, GRP, 1], F32, tag="p1")
            nc.vector.tensor_mul(p1_t, p3_t[:, :, 0:1], p3_t[:, :, 1:2])
            c_t = small.tile([P, GRP, 1], F32, tag="c")
            nc.vector.tensor_mul(c_t, p1_t, p3_t[:, :, 2:3])
            for j in range(GRP):
                n = g * GRP + j
                nc.vector.tensor_scalar_mul(chunk(n), chunk(n), c_t[:, j, :])
        else:
            for j in range(GRP):
                n = g * GRP + j
                u1_t = small.tile([P, L], F32, tag="u1")
                nc.vector.tensor_copy(u1_t, uj_ps[:, j, :])
                c_t = small.tile([P, 1], F32, tag="c")
                nc.vector.memset(c_t, 1.0)
                for i in range(L):
                    # c = c * (1 + u_i) + beta_i
                    nc.vector.scalar_tensor_tensor(
                        out=c_t,
                        in0=c_t,
                        scalar=u1_t[:, i : i + 1],
                        in1=beta_sb[:, i : i + 1],
                        op0=mybir.AluOpType.mult,
                        op1=mybir.AluOpType.add,
                    )
                # out = x * c + B_L
                nc.vector.scalar_tensor_tensor(
                    out=chunk(n),
                    in0=chunk(n),
                    scalar=c_t,
                    in1=bl_sb,
                    op0=mybir.AluOpType.mult,
                    op1=mybir.AluOpType.add,
                )


def _split_multi_waits(nc):
    """This walrus build allows only one sem wait on several instruction
    structs (e.g. self-loading Matmult). Move extra waits onto preceding
    same-engine NOPs; engine FIFO order makes this equivalent."""
    n = 0
    for fn in nc.m.functions:
        for bb in fn.blocks:
            out = []
            for inst in bb.instructions:
                si = inst.sync_info
                if si is not None and si.on_wait and len(si.on_wait) > 1:
                    waits = list(si.on_wait)
                    for w in waits[:-1]:
                        n += 1
                        out.append(
                            mybir.InstNoOp(
                                name=f"nopw-{n}-{inst.name}",
                                engine=inst.engine,
                                sync_info=mybir.SyncInfo(
                                    on_wait=[w], on_update=[]
                                ),
                                bass_nofuse=True,
                            )
                        )
                    inst.sync_info = mybir.SyncInfo(
                        on_wait=[waits[-1]], on_update=list(si.on_update)
                    )
                out.append(inst)
            bb.instructions = out


_CACHE = {}


def _get_nc(with_bias: bool, loop_n: int = 1, mode: str = "full"):
    key = (with_bias, loop_n, mode)
    if key not in _CACHE:
        _patch_tile_drain()
        nc = _build(with_bias, loop_n, mode)
        _split_multi_waits(nc)
        _CACHE[key] = nc
    return _CACHE[key]


def kernel(input, W, b, **run_kwargs):
    input = np.ascontiguousarray(np.asarray(input, dtype=np.float32))
    W = np.asarray(W, dtype=np.float32)
    b = np.asarray(b, dtype=np.float32)
    assert input.shape == (B, D) and W.shape == (L, D) and b.shape == (L, D)

    with_bias = bool(np.any(b))
    nc = _get_nc(with_bias)

    wt = np.ascontiguousarray(W.T)  # [D, L]
    in_maps = []
    for i in range(NCORES):
        m = {"x": input[i * R : (i + 1) * R], "wt": wt}
        if with_bias:
            # B_i = sum_{j<i} b_j ; beta_i = B_i . w_i ; B_L = sum_j b_j
            Bpre = np.concatenate(
                [np.zeros((1, D), np.float32), np.cumsum(b, axis=0)[:-1]], axis=0
            )
            m["bl"] = b.sum(axis=0, dtype=np.float32).reshape(1, D)
            m["beta"] = np.einsum("ld,ld->l", Bpre, W).astype(np.float32).reshape(1, L)
        in_maps.append(m)

    res = run_bass_kernel_spmd(
        nc, in_maps, core_ids=list(range(NCORES)), **run_kwargs
    )
    out = np.concatenate([res.results[i]["y"] for i in range(NCORES)], axis=0)
    if run_kwargs:
        return out, res
    return out


# revision 4
# speedup vs baseline: 1.2597x; 1.2597x over previous
"""Trainium2 Bass kernel for nn_CrossNetwork (DCN-v1 cross network), v3.

Math: reference computes x_{i+1} = input * (x_i . w_i) + x_i + b_i, L=6 layers.
Writing x_i = input * c_i + B_i with B_i = sum_{j<i} b_j (a constant row
vector) and c_i a per-row scalar, the recursion collapses to
    u_i    = input . w_i                     (per row, one tall-skinny matmul)
    beta_i = B_i . w_i                       (host-computed constants)
    c_{i+1} = c_i * (1 + u_i) + beta_i ; c_0 = 1
    out    = input * c_L + B_L
For the b == 0 case this is out = input * prod_i(1 + u_i).

v3: software-pipelined double buffering across loop passes with monolithic
8MB load/store DMAs (ring order L_A S_B L_B S_A -> long same-direction runs,
minimal read/write turnaround, near-peak HBM BW); bf16 matmuls (1 cyc/row);
PSUM->SBUF copies on ACT, scales + scalar chain on DVE, no gpsimd.
"""

import numpy as np

import concourse.bass as bass
import concourse.mybir as mybir
import concourse.tile as tile
from concourse.bass_utils import run_bass_kernel_spmd
from concourse.masks import make_identity
from concourse.vector_clock import ScopedClock

F32 = mybir.dt.float32
BF16 = mybir.dt.bfloat16

B, D, L = 16384, 1024, 6
NCORES = 8
R = B // NCORES  # rows per core (2048)
P = 128
NCH = R // P  # chunks of 128 rows per core (16)
KB = D // P  # 128-wide k blocks (8)
GRP = 4  # chunks per psum accumulation group
NG = NCH // GRP  # 4 groups

MM_DT = BF16  # matmul operand dtype (bf16: 1 cyc/row vs fp32's 4)


def _patch_tile_drain():
    """This walrus build rejects >1 sem wait on a CTRL (Drain) instruction.

    Tile's kernel-tail drain waits on every sem domain at once; split it into
    chained single-wait drains.
    """
    if getattr(tile.TileContext, "_drain_patched", False):
        return

    def _drain_and_barrier(self, tick_clock, wait_clock):
        gc = tick_clock.global_clock
        entries = [(proc, t) for proc, t in enumerate(gc) if t > 0]
        if not entries:
            self.nc.sync.drain()
        for proc, t in entries:
            sub = ScopedClock()
            sub.require_at_least(None, proc, t)
            drain_inst = self.nc.sync.drain()
            wait_clock.add_sem_waits(drain_inst.ins, sub)

        self.nc.all_engine_barrier()
        assert self.sems is not None
        popped = self.nc._tile_sem_poison_stack.pop()
        assert popped is self._sem_poison
        self.nc.clear_and_free_semaphores(list(self.sems.allocated().values()))

    tile.TileContext._drain_and_barrier = _drain_and_barrier
    tile.TileContext._drain_patched = True


def _build(with_bias: bool, loop_n: int = 1, mode: str = "full"):
    nc = bass.Bass("TRN2")
    x_d = nc.dram_tensor("x", [R, D], F32, kind="ExternalInput")
    wt_d = nc.dram_tensor("wt", [D, L], F32, kind="ExternalInput")
    if with_bias:
        bl_d = nc.dram_tensor("bl", [1, D], F32, kind="ExternalInput")
        beta_d = nc.dram_tensor("beta", [1, L], F32, kind="ExternalInput")
    y_d = nc.dram_tensor("y", [R, D], F32, kind="ExternalOutput")

    # row = p*NCH + n ; whole-slab load: per-partition 64KB contiguous
    xv = x_d.rearrange("(p n) d -> p n d", p=P)  # [128, NCH, D]
    yv = y_d.rearrange("(p n) d -> p n d", p=P)
    wtv = wt_d.rearrange("(k p) s -> p k s", p=P)  # [128, KB, L]

    with tile.TileContext(nc) as tc:
        with (
            tc.tile_pool(name="consts", bufs=1) as consts,
            tc.tile_pool(name="xbuf", bufs=1) as xpool,
            tc.tile_pool(name="xt", bufs=6) as xtpool,
            tc.tile_pool(name="small", bufs=2 * GRP) as small,
            tc.tile_pool(name="pxt", bufs=4, space="PSUM") as pxt,
            tc.tile_pool(name="pu", bufs=2, space="PSUM") as pu,
        ):
            ident = consts.tile([P, P], F32)
            make_identity(nc, ident)
            ident6 = consts.tile([L, L], F32)
            make_identity(nc, ident6)
            wt_sb = consts.tile([P, KB, L], F32)
            nc.sync.dma_start(out=wt_sb, in_=wtv)
            wt_mm = wt_sb
            if MM_DT is not F32:
                wt_mm = consts.tile([P, KB, L], MM_DT, name="wt_mm")
                nc.vector.tensor_copy(wt_mm, wt_sb)
            if with_bias:
                bl_sb = consts.tile([P, D], F32)
                nc.sync.dma_start(
                    out=bl_sb,
                    in_=bass.AP(tensor=bl_d, offset=0, ap=[[0, P], [1, D]]),
                )
                beta_sb = consts.tile([P, L], F32)
                nc.sync.dma_start(
                    out=beta_sb,
                    in_=bass.AP(tensor=beta_d, offset=0, ap=[[0, P], [1, L]]),
                )
            bl = locals().get("bl_sb")
            beta = locals().get("beta_sb")

            args = (nc, tc, xtpool, small, pxt, pu, ident, ident6, wt_mm,
                    bl, beta, with_bias)

            if loop_n > 1:
                xa = xpool.tile([P, NCH, D], F32, tag="xa", name="xa")
                xb = xpool.tile([P, NCH, D], F32, tag="xb", name="xb")
                # pipeline prologue: pre-fill B so the first S_B is a
                # well-defined (if unscaled) store; overwritten in-iteration.
                nc.sync.dma_start(out=xb, in_=xv)
                with tc.For_i(0, loop_n, 1):
                    # ring: L_A  S_B  L_B  S_A  (two passes per trip)
                    nc.sync.dma_start(out=xa, in_=xv)
                    if mode != "dma":
                        _compute(xa, *args)
                    nc.sync.dma_start(out=yv, in_=xb)
                    nc.sync.dma_start(out=xb, in_=xv)
                    if mode != "dma":
                        _compute(xb, *args)
                    nc.sync.dma_start(out=yv, in_=xa)
            else:
                # single-pass build (the graded kernel() path): per-group
                # loads/stores so compute overlaps the DMA stream.
                xa = xpool.tile([P, NCH, D], F32, tag="xa", name="xa")
                for g in range(NG):
                    nc.sync.dma_start(
                        out=xa[:, g * GRP : (g + 1) * GRP, :],
                        in_=xv[:, g * GRP : (g + 1) * GRP, :],
                    )
                for g in range(NG):
                    _compute(xa, *args, groups=[g])
                    nc.sync.dma_start(
                        out=yv[:, g * GRP : (g + 1) * GRP, :],
                        in_=xa[:, g * GRP : (g + 1) * GRP, :],
                    )
    return nc


def _compute(x_sb, nc, tc, xtpool, small, pxt, pu, ident, ident6, wt_mm,
             bl_sb, beta_sb, with_bias, groups=None):
    def chunk(n):  # chunk n -> AP [P, D]
        return x_sb[:, n, :]

    for g in (range(NG) if groups is None else groups):
        # U^T[6, 512] accumulated over k blocks; stationary weights
        # are only 6 columns so LDWEIGHTS is trivial.
        # Two phases of (16 transposes -> 4 matmuls) so the PE FIFO never
        # stalls waiting on a PSUM->SBUF copy: copy k completes while the
        # transposes for k+1..k+3 stream.
        ut_ps = pu.tile([L, GRP * P], F32, tag="u", name=f"ut{g}")
        for ph in range(2):
            xts = []
            for kk in range(KB // 2):
                k = ph * (KB // 2) + kk
                pxt_t = pxt.tile([P, GRP * P], F32, tag="pxt")
                for j in range(GRP):
                    nc.tensor.transpose(
                        pxt_t[:, j * P : (j + 1) * P],
                        chunk(g * GRP + j)[:, k * P : (k + 1) * P],
                        ident,
                    )
                xt_t = xtpool.tile([P, GRP * P], MM_DT, tag="xt")
                nc.scalar.copy(xt_t, pxt_t)
                xts.append((k, xt_t))
            for k, xt_t in xts:
                nc.tensor.matmul(
                    ut_ps[:],
                    wt_mm[:, k, :],
                    xt_t[:],
                    start=(k == 0),
                    stop=(k == KB - 1),
                )
        # 1 + U^T on DVE while copying PSUM->SBUF, then transpose
        # [6,128] blocks back to row-major [128,6] per chunk.
        u1t_t = xtpool.tile([L, GRP * P], F32, tag="u1t")
        nc.vector.tensor_scalar_add(u1t_t, ut_ps, 1.0)
        uj_ps = pu.tile([P, GRP, L], F32, tag="uj", name=f"uj{g}")
        for j in range(GRP):
            nc.tensor.transpose(
                uj_ps[:, j, :],
                u1t_t[:, j * P : (j + 1) * P],
                ident6,
            )
        if not with_bias:
            # group-batched: c = prod over the 6 (1+u_i) for all 4 chunks
            u1g_t = small.tile([P, GRP, L], F32, tag="u1g")
            nc.vector.tensor_copy(u1g_t, uj_ps)
            p3_t = small.tile([P, GRP, 3], F32, tag="p3")
            nc.vector.tensor_mul(p3_t, u1g_t[:, :, 0:3], u1g_t[:, :, 3:6])
            p1_t = small.tile([P, GRP, 1], F32, tag="p1")
            nc.vector.tensor_mul(p1_t, p3_t[:, :, 0:1], p3_t[:, :, 1:2])
            c_t = small.tile([P, GRP, 1], F32, tag="c")
            nc.vector.tensor_mul(c_t, p1_t, p3_t[:, :, 2:3])
            for j in range(GRP):
                n = g * GRP + j
                nc.vector.tensor_scalar_mul(chunk(n), chunk(n), c_t[:, j, :])
        else:
            for j in range(GRP):
                n = g * GRP + j
                u1_t = small.tile([P, L], F32, tag="u1")
                nc.vector.tensor_copy(u1_t, uj_ps[:, j, :])
                c_t = small.tile([P, 1], F32, tag="c")
                nc.vector.memset(c_t, 1.0)
                for i in range(L):
                    # c = c * (1 + u_i) + beta_i
                    nc.vector.scalar_tensor_tensor(
                        out=c_t,
                        in0=c_t,
                        scalar=u1_t[:, i : i + 1],
                        in1=beta_sb[:, i : i + 1],
                        op0=mybir.AluOpType.mult,
                        op1=mybir.AluOpType.add,
                    )
                # out = x * c + B_L
                nc.vector.scalar_tensor_tensor(
                    out=chunk(n),
                    in0=chunk(n),
                    scalar=c_t,
                    in1=bl_sb,
                    op0=mybir.AluOpType.mult,
                    op1=mybir.AluOpType.add,
                )


def _split_multi_waits(nc):
    """This walrus build allows only one sem wait on several instruction
    structs (e.g. self-loading Matmult). Move extra waits onto preceding
    same-engine NOPs; engine FIFO order makes this equivalent."""
    n = 0
    for fn in nc.m.functions:
        for bb in fn.blocks:
            out = []
            for inst in bb.instructions:
                si = inst.sync_info
                if si is not None and si.on_wait and len(si.on_wait) > 1:
                    waits = list(si.on_wait)
                    for w in waits[:-1]:
                        n += 1
                        out.append(
                            mybir.InstNoOp(
                                name=f"nopw-{n}-{inst.name}",
                                engine=inst.engine,
                                sync_info=mybir.SyncInfo(
                                    on_wait=[w], on_update=[]
                                ),
                                bass_nofuse=True,
                            )
                        )
                    inst.sync_info = mybir.SyncInfo(
                        on_wait=[waits[-1]], on_update=list(si.on_update)
                    )
                out.append(inst)
            bb.instructions = out


_CACHE = {}


def _get_nc(with_bias: bool, loop_n: int = 1, mode: str = "full"):
    key = (with_bias, loop_n, mode)
    if key not in _CACHE:
        _patch_tile_drain()
        nc = _build(with_bias, loop_n, mode)
        _split_multi_waits(nc)
        _CACHE[key] = nc
    return _CACHE[key]


def kernel(input, W, b, **run_kwargs):
    input = np.ascontiguousarray(np.asarray(input, dtype=np.float32))
    W = np.asarray(W, dtype=np.float32)
    b = np.asarray(b, dtype=np.float32)
    assert input.shape == (B, D) and W.shape == (L, D) and b.shape == (L, D)

    with_bias = bool(np.any(b))
    nc = _get_nc(with_bias)

    wt = np.ascontiguousarray(W.T)  # [D, L]
    in_maps = []
    for i in range(NCORES):
        m = {"x": input[i * R : (i + 1) * R], "wt": wt}
        if with_bias:
            # B_i = sum_{j<i} b_j ; beta_i = B_i . w_i ; B_L = sum_j b_j
            Bpre = np.concatenate(
                [np.zeros((1, D), np.float32), np.cumsum(b, axis=0)[:-1]], axis=0
            )
            m["bl"] = b.sum(axis=0, dtype=np.float32).reshape(1, D)
            m["beta"] = np.einsum("ld,ld->l", Bpre, W).astype(np.float32).reshape(1, L)
        in_maps.append(m)

    res = run_bass_kernel_spmd(
        nc, in_maps, core_ids=list(range(NCORES)), **run_kwargs
    )
    out = np.concatenate([res.results[i]["y"] for i in range(NCORES)], axis=0)
    if run_kwargs:
        return out, res
    return out
